# revision 15
# baseline (speedup 1.0000x reference)
"""GQA attention block (RoPE + causal softmax + out-projection) on 8 TRN2 cores.

Problem: q (2, 2048, 1024) 16 heads, k/v (2, 2048, 256) 4 kv heads (GQA rep 4),
causal attention, out @ w_out (1024, 1024).

Sharding: core c = (batch b = c//4, kv group = c%4). Each core computes its 4
q-heads x full T attention against its kv head, then the partial projection
X_heads @ w_out[head_rows, :]; the host sums the 4 partials per batch.

Layout: everything is computed transposed (channels on partitions, sequence on
the free axis):
  - S^T = K^T-block (64,128) stationary @ Q^T (64, q-cols) moving (k on psum
    partitions); q groups are 1024 wide (2 psum banks; matmuls write per-bank)
  - causal mask = one extra 128-row PE matmul (identity stationary, mask
    moving) accumulating -240 onto the diagonal strip -- no vector writes in
    the QK->exp chain
  - P^T = exp(S^T/8) as ONE wide ACT instruction per (head, kb) reading both
    psum banks; no row-max (logits are O(1)) and no P normalization
  - O^T accumulates [V | 1] (128k, 65) stationary @ P^T moving, so the softmax
    denominator falls out as psum row 64; normalization = reciprocal (DVE) +
    partition_broadcast (Pool) + one multiply per (head, group)
  - projection: w-chunk (128c, 128n) stationary @ X^T (128c, cols) moving,
    interleaved into the NEXT group's attention so PE never idles; psum copied
    to SBUF bf16 (split DVE/ACT) and DMA'd as out^T; host transposes+sums.
RoPE runs on-chip: rotate_half is a signed-permutation matmul, the sin/cos
combine is 2 DVE ops + 1 Pool op per 512-chunk; later chunks are interleaved
into earlier groups' attention. Engines: PE ~80us, ACT ~80us (exp), DVE ~45us,
Pool ~15us; span target ~90-100us.
"""

import sys

if "/opt/trn_rl_repo" not in sys.path:
    sys.path.insert(0, "/opt/trn_rl_repo")

import numpy as np

B, T, D, NH, NKV, HD = 2, 2048, 1024, 16, 4, 64
HC = NH // NKV          # q heads per core = 4
CD = HC * HD            # per-core channel dim = 256
KVD = HD                # per-core kv channel dim = 64
NCORES = 8
QB = 128                # k block
GW = 1024               # q group width (2 psum banks)
NGB = T // GW           # 2 groups
NKB = T // QB           # 16 k blocks
MASK = -240.0           # pre-scale additive mask; exp(-240/8) = exp(-30) ~ 1e-13
DEBUG = False           # add intermediate dumps (xT, S, PT) as extra outputs

_cache: dict = {}


def _tables():
    if "tables" in _cache:
        return _cache["tables"]
    p = np.arange(128)
    t = np.arange(T)
    ang = t[None, :] / (10000.0 ** ((p[:, None] % 32) / 32.0))
    cosT = np.cos(ang).astype(np.float32)
    sinT = np.sin(ang).astype(np.float32)

    rotP = np.zeros((128, 128), np.float32)
    for base in (0, 64):
        for i in range(32):
            rotP[base + 32 + i, base + i] = -1.0   # out[i] = -x[i+32]
            rotP[base + i, base + 32 + i] = 1.0    # out[i+32] = x[i]

    kk = np.arange(QB)
    maskT = np.where(kk[:, None] <= kk[None, :], 0.0, MASK).astype(np.float32)
    ident = np.eye(128, dtype=np.float32)
    _cache["tables"] = (cosT, sinT, rotP, maskT, ident)
    return _cache["tables"]


def _build():
    import concourse.tile as tile
    from concourse import bacc, mybir

    f32 = mybir.dt.float32
    bf16 = mybir.dt.bfloat16
    Exp = mybir.ActivationFunctionType.Exp

    nc = bacc.Bacc("TRN2", target_bir_lowering=False, debug=False,
                   num_devices=NCORES)

    d_qT = nc.dram_tensor("qT", [CD, T], bf16, kind="ExternalInput")
    d_kT = nc.dram_tensor("kT", [KVD, T], bf16, kind="ExternalInput")
    d_vaug = nc.dram_tensor("vaug", [128, NKB * (HD + 1)], bf16,
                            kind="ExternalInput")
    d_w = nc.dram_tensor("w", [CD, D], bf16, kind="ExternalInput")
    d_cosT = nc.dram_tensor("cosT", [128, T], bf16, kind="ExternalInput")
    d_sinT = nc.dram_tensor("sinT", [128, T], bf16, kind="ExternalInput")
    d_rotP = nc.dram_tensor("rotP", [128, 128], bf16, kind="ExternalInput")
    d_maskT = nc.dram_tensor("maskT", [QB, QB], bf16, kind="ExternalInput")
    d_ident = nc.dram_tensor("ident", [128, 128], bf16, kind="ExternalInput")
    d_outT = nc.dram_tensor("outT", [D, T], bf16, kind="ExternalOutput")
    if DEBUG:
        d_dbg_xT = nc.dram_tensor("dbg_xT", [256, T], bf16,
                                  kind="ExternalOutput")
        d_dbg_S = nc.dram_tensor("dbg_S", [128, GW], mybir.dt.float32,
                                 kind="ExternalOutput")
        d_dbg_PT = nc.dram_tensor("dbg_PT", [128, GW], bf16,
                                  kind="ExternalOutput")
        d_dbg_den = nc.dram_tensor("dbg_den", [1, 8 * GW], mybir.dt.float32,
                                   kind="ExternalOutput")

    with tile.TileContext(nc) as tc:
        with (
            tc.tile_pool(name="consts", bufs=1) as consts,
            tc.tile_pool(name="data", bufs=1) as data,
            tc.tile_pool(name="pt", bufs=4) as ptp,
            tc.tile_pool(name="small", bufs=4) as small,
            tc.tile_pool(name="psS", bufs=2, space="PSUM") as psS,
            tc.tile_pool(name="psO", bufs=1, space="PSUM") as psO,
        ):
            cosT = consts.tile([128, T], bf16)
            sinT = consts.tile([128, T], bf16)
            rotP = consts.tile([128, 128], bf16)
            maskT = consts.tile([QB, QB], bf16)
            ident = consts.tile([128, 128], bf16)
            qT = [data.tile([128, T], bf16, name=f"qT{i}", tag=f"qT{i}")
                  for i in range(2)]
            kT2 = data.tile([128, T], bf16, tag="kT2")
            vaug = data.tile([128, NKB, HD + 1], bf16, tag="vaug")
            w = [data.tile([128, D], bf16, name=f"w{i}", tag=f"w{i}")
                 for i in range(2)]
            xT = [data.tile([128, T], bf16, name=f"xT{i}", tag=f"xT{i}")
                  for i in range(2)]
            if DEBUG:
                dbgS = data.tile([128, GW], f32, tag="dbgS")
                den_keep = data.tile([1, 8, GW], f32, tag="den_keep")

            out_sb = data.tile([128, D // 128, T], bf16, tag="out_sb")

            # input DMAs spread across engine queues (descriptor gen is
            # ~20ns/partition-row on the issuing engine); ordered so rope
            # chunk 0 deps land first
            nc.gpsimd.dma_start(rotP[:], d_rotP[:])
            nc.gpsimd.dma_start(maskT[:], d_maskT[:])
            nc.gpsimd.dma_start(ident[:], d_ident[:])
            nc.gpsimd.dma_start(qT[0][:, 0:1024], d_qT[0:128, 0:1024])
            nc.gpsimd.dma_start(kT2[64:128, :], d_kT[:])
            nc.gpsimd.dma_start(w[0][:], d_w[0:128, :])
            nc.gpsimd.dma_start(w[1][:], d_w[128:256, :])
            nc.sync.dma_start(cosT[:], d_cosT[:])
            nc.sync.dma_start(qT[0][:, 1024:2048], d_qT[0:128, 1024:2048])
            nc.scalar.dma_start(sinT[:], d_sinT[:])
            nc.scalar.dma_start(kT2[0:64, :], d_kT[:])
            nc.scalar.dma_start(vaug[:].rearrange("p n m -> p (n m)"),
                                d_vaug[:])
            nc.scalar.dma_start(qT[1][:], d_qT[128:256, :])

            # ---- RoPE as interleavable item lists (512-col chunks) ----
            def rope_items(dst, c):
                sl = slice(512 * c, 512 * (c + 1))

                def mm():
                    rot = psS.tile([128, 512], f32, name="rot", tag="S")
                    nc.tensor.matmul(rot[:], rotP[:], dst[:, sl],
                                     start=True, stop=True)
                    return rot

                box = {}

                def i0():
                    box["rot"] = mm()

                def i1():
                    nc.vector.tensor_mul(box["rot"][:], box["rot"][:],
                                         sinT[:, sl])

                def i2():
                    nc.gpsimd.tensor_mul(dst[:, sl], dst[:, sl], cosT[:, sl])

                def i3():
                    nc.vector.tensor_add(dst[:, sl], dst[:, sl], box["rot"][:])

                return [i0, i1, i2, i3]

            # upfront rope: qT0 cols 0:1024 and kT2 cols 0:1024 (group 0 deps)
            pre = []
            for c in (0, 1):
                pre.append(rope_items(qT[0], c))
            for c in (0, 1):
                pre.append(rope_items(kT2, c))
            # PE rot matmuls first (back-to-back), then the vector chains
            for items in pre:
                items[0]()
            # warmup matmuls keep PE busy (HAM ramp) while DVE runs rope
            for i in range(24):
                wt = psS.tile([128, 128], f32, name="warm", tag="S")
                nc.tensor.matmul(wt[:], rotP[:], rotP[:], start=True,
                                 stop=True)
            for items in pre:
                for it in items[1:]:
                    it()

            # background queue: (key, thunk) drained into the attention loops
            bg = []

            def enq(key, thunks):
                for th in thunks:
                    bg.append((key, th))

            def drain(n):
                for _ in range(min(n, len(bg))):
                    bg.pop(0)[1]()

            def drain_until(key):
                while any(k == key for k, _ in bg):
                    bg.pop(0)[1]()

            for c in (0, 1):
                enq("qT1a", rope_items(qT[1], c))
            for c in (2, 3):
                enq("kT2b", rope_items(kT2, c))
            for c in (2, 3):
                enq("qT0b", rope_items(qT[0], c))
            for c in (2, 3):
                enq("qT1b", rope_items(qT[1], c))

            need = {(0, 0): [], (0, 1): ["qT1a"],
                    (1, 0): ["kT2b", "qT0b"], (1, 1): ["qT1b"]}

            def proj_items(g):
                qlo = g * GW
                out = []
                for n in range(D // 128):
                    box = {}

                    def i0(n=n, box=box):
                        pr = psS.tile([128, GW], f32, name="pr", tag="S")
                        for cc in range(2):
                            nc.tensor.matmul(
                                pr[:, 0:512],
                                w[cc][:, n * 128:(n + 1) * 128],
                                xT[cc][:, qlo:qlo + 512],
                                start=(cc == 0), stop=(cc == 1))
                        box["pr"] = pr

                    def i1(n=n, box=box):
                        pr = box["pr"]
                        for cc in range(2):
                            nc.tensor.matmul(
                                pr[:, 512:1024],
                                w[cc][:, n * 128:(n + 1) * 128],
                                xT[cc][:, qlo + 512:qlo + 1024],
                                start=(cc == 0), stop=(cc == 1))

                    def i2(n=n, box=box, g=g):
                        pr = box["pr"]
                        nc.vector.tensor_copy(out_sb[:, n, qlo:qlo + 512],
                                              pr[:, 0:512])
                        nc.scalar.copy(out_sb[:, n, qlo + 512:qlo + 1024],
                                       pr[:, 512:1024])
                        eng = nc.sync if n % 2 == 0 else nc.gpsimd
                        eng.dma_start(
                            d_outT[n * 128:(n + 1) * 128, qlo:qlo + GW],
                            out_sb[:, n, qlo:qlo + GW])

                    out += [i0, i1, i2]
                return out

            # ---- attention ----
            for g in range(NGB):
                qlo = g * GW
                nkb = (g + 1) * (GW // QB)
                for hp in range(2):
                    for key in need[(g, hp)]:
                        drain_until(key)
                    oTs = [psO.tile([HD + 1, GW], f32, name=f"oT{hh}",
                                    tag=f"oT{hh}") for hh in range(2)]
                    for kb in range(nkb):
                        diag = kb >= (GW // QB) * g
                        cs = QB * (kb - (GW // QB) * g) if diag else 0
                        for hh in range(2):
                            qoff = 64 * hh
                            S = psS.tile([128, GW], f32, name="S", tag="S")
                            kst = kT2[qoff:qoff + 64,
                                      kb * QB:(kb + 1) * QB]
                            if cs < 512:
                                nc.tensor.matmul(
                                    S[:, cs:512], kst,
                                    qT[hp][qoff:qoff + 64,
                                           qlo + cs:qlo + 512],
                                    start=True, stop=not diag,
                                    skip_group_check=True)
                                nc.tensor.matmul(
                                    S[:, 512:1024], kst,
                                    qT[hp][qoff:qoff + 64,
                                           qlo + 512:qlo + 1024],
                                    start=True, stop=True,
                                    skip_group_check=True)
                            else:
                                nc.tensor.matmul(
                                    S[:, cs:1024], kst,
                                    qT[hp][qoff:qoff + 64,
                                           qlo + cs:qlo + 1024],
                                    start=True, stop=False,
                                    skip_group_check=True)
                            if diag:
                                # accumulate the causal mask onto the diagonal
                                # 128-strip on the PE itself
                                nc.tensor.matmul(
                                    S[:, cs:cs + QB], ident[:], maskT[:],
                                    start=False, stop=True,
                                    skip_group_check=True)
                            PT = ptp.tile([128, GW], bf16, name="PT",
                                          tag="PT")
                            nc.scalar.activation(PT[:, cs:], S[:, cs:], Exp,
                                                 scale=0.125)
                            if DEBUG and (g, hp, kb, hh) == (0, 0, 1, 0):
                                nc.vector.tensor_copy(dbgS[:, cs:], S[:, cs:])
                                nc.sync.dma_start(d_dbg_S[:], dbgS[:])
                                nc.sync.dma_start(d_dbg_PT[:], PT[:])
                            segs = ([(cs, 512), (512, 1024)] if cs < 512
                                    else [(cs, 1024)])
                            for a, b in segs:
                                nc.tensor.matmul(
                                    oTs[hh][:, a:b], vaug[:, kb, :],
                                    PT[:, a:b],
                                    start=(kb == 0), stop=(kb == nkb - 1),
                                    skip_group_check=True)
                            drain(2)
                    for hh in range(2):
                        # den must round-trip through SBUF: the custom-DVE
                        # reciprocal bit-tricks misread PSUM's raw accumulator
                        # format
                        den_raw = small.tile([1, GW], f32, tag="denr")
                        nc.vector.tensor_copy(den_raw[:],
                                              oTs[hh][HD:HD + 1, :])
                        den = small.tile([1, GW], f32, tag="den")
                        nc.vector.reciprocal_approx_fast(den[:], den_raw[:])
                        bcs = small.tile([64, GW], f32, tag="bcs", bufs=2)
                        nc.gpsimd.partition_broadcast(bcs[:], den[:])
                        nc.vector.tensor_mul(
                            xT[hp][64 * hh:64 * hh + 64, qlo:qlo + GW],
                            oTs[hh][:HD, :], bcs[:])
                        if DEBUG:
                            idx = g * 4 + hp * 2 + hh
                            nc.vector.tensor_copy(
                                den_keep[0:1, idx, :],
                                oTs[hh][HD:HD + 1, :])
                enq(f"proj{g}", proj_items(g))

            while bg:
                bg.pop(0)[1]()
            if DEBUG:
                nc.sync.dma_start(d_dbg_xT[0:128, :], xT[0][:])
                nc.sync.dma_start(d_dbg_xT[128:256, :], xT[1][:])
                nc.sync.dma_start(d_dbg_den[:], den_keep[0:1, :, :])

    nc.finalize()
    return nc


def _get_nc():
    if "nc" not in _cache:
        _cache["nc"] = _build()
    return _cache["nc"]


def _in_maps(q, k, v, w_out):
    import ml_dtypes
    bf = ml_dtypes.bfloat16
    cosT, sinT, rotP, maskT, ident = _tables()
    ones = np.ones((T, 1), np.float32)
    maps = []
    for c in range(NCORES):
        b, kv = divmod(c, NKV)
        maps.append({
            "qT": np.ascontiguousarray(q[b, :, kv * CD:(kv + 1) * CD].T).astype(bf),
            "kT": np.ascontiguousarray(k[b, :, kv * KVD:(kv + 1) * KVD].T).astype(bf),
            "vaug": np.ascontiguousarray(
                np.concatenate([v[b, :, kv * KVD:(kv + 1) * KVD], ones], 1)
                .reshape(NKB, 128, HD + 1).transpose(1, 0, 2)
                .reshape(128, NKB * (HD + 1))).astype(bf),
            "w": np.ascontiguousarray(w_out[kv * CD:(kv + 1) * CD, :]).astype(bf),
            "cosT": cosT.astype(bf), "sinT": sinT.astype(bf),
            "rotP": rotP.astype(bf), "maskT": maskT.astype(bf),
            "ident": ident.astype(bf),
        })
    return maps


def _run(q, k, v, w_out, trace=False):
    from concourse.bass_utils import run_bass_kernel_spmd

    nc = _get_nc()
    res = run_bass_kernel_spmd(nc, _in_maps(q, k, v, w_out),
                               core_ids=list(range(NCORES)), trace=trace)
    out = np.zeros((B, T, D), np.float32)
    for c in range(NCORES):
        out[c // NKV] += res.results[c]["outT"].T.astype(np.float32)
    return out, res


def kernel(q, k, v, w_out):
    out, _ = _run(np.asarray(q), np.asarray(k), np.asarray(v),
                  np.asarray(w_out))
    return out
